# revision 7
# baseline (speedup 1.0000x reference)
"""Trainium2 Bass kernel for nn_KalmanFilter (B=1024, T=256, S=16, M=4).

Math: the covariance P and Kalman gains K_t never touch the observations,
and P0 = diag(std^2) is identical for every batch element, so K_t (hence
A_t = F(I - K_t H), B_t = F K_t) are shared across the batch.  The filter
collapses to the linear time-varying recurrence

    m_0 = F x0,   m_{t+1} = A_t m_t + B_t u_t,   out_t = H m_t .

Condensing time into Nb blocks of Tb steps (Tb*M = 128):

    out_block_k = U_k^T G_k^T + m_k^T W_k^T        (per-block, batched)
    m_k         = MM_k u_all + Phi(k*Tb, 0) m_0    (block-boundary states)

where U_k is the observation block laid out [Tb*M, Bc].  All the small
shared matrices (G_k, W_k, MM, Phi-stack) are data-independent and computed
on the host in float64; the device does only dense 128x128-ish matmuls,
data-parallel over the batch on 8 cores (128 batch rows per core).
"""

import numpy as np

S, M, T, B = 16, 4, 256, 1024
Tb = 32
Nb = T // Tb          # 8
TbM = Tb * M          # 128
TM = T * M            # 1024
NCORES = 8
BC = B // NCORES      # 128


# ---------------------------------------------------------------- host math
def _gains(F, H, Q, R, x0, std):
    """Shared A_t [T-1,S,S], B_t [T-1,S,M] and m0 [S] in float64."""
    F, H, Q, R = (np.asarray(a, np.float64) for a in (F, H, Q, R))
    x0 = np.asarray(x0, np.float64)
    P = np.diag(np.asarray(std, np.float64) ** 2)
    I = np.eye(S)
    m0 = F @ x0
    P = F @ P @ F.T + Q
    A = np.zeros((T - 1, S, S))
    Bm = np.zeros((T - 1, S, M))
    for s in range(T - 1):
        Sm = H @ P @ H.T + R
        K = P @ H.T @ np.linalg.inv(Sm)
        IKH = I - K @ H
        A[s] = F @ IKH
        Bm[s] = F @ K
        P = IKH @ P @ IKH.T + K @ R @ K.T
        P = F @ P @ F.T + Q
    return A, Bm, m0


def _block_mats(A, Bm, H):
    """G_k^T [Nb,TbM,TbM], W_k^T [Nb,S,TbM], MM [Nb*S,TM], PhiStackT [S,Nb*S]."""
    H = np.asarray(H, np.float64)
    Gt = np.zeros((Nb, TbM, TbM))
    Wt = np.zeros((Nb, S, TbM))
    MM = np.zeros((Nb * S, TM))
    PhiStackT = np.zeros((S, Nb * S))

    Phi_g = np.eye(S)            # Phi(t0, 0)
    Vg = np.zeros((S, 0))        # [S, t0*M]: Phi(t0, s+1) B_s for s < t0
    for k in range(Nb):
        t0 = k * Tb
        PhiStackT[:, k * S:(k + 1) * S] = Phi_g.T
        MM[k * S:(k + 1) * S, :t0 * M] = Vg
        Phi = np.eye(S)
        V = np.zeros((S, 0))     # within-block propagated B columns
        for dt in range(Tb):
            t = t0 + dt
            Wt[k][:, dt * M:(dt + 1) * M] = (H @ Phi).T
            if dt > 0:
                Gt[k][: dt * M, dt * M:(dt + 1) * M] = (H @ V).T
            if t <= T - 2:
                V = np.concatenate([A[t] @ V, Bm[t]], 1)
                Phi = A[t] @ Phi
        if t0 + Tb <= T - 1:
            Vg = np.concatenate([Phi @ Vg, V], 1)
            Phi_g = Phi @ Phi_g
    return Gt, Wt, MM, PhiStackT


def _device_params(F, H, Q, R, x0, std):
    A, Bm, m0 = _gains(F, H, Q, R, x0, std)
    Gt, Wt, MM, PhiStackT = _block_mats(A, Bm, H)
    f32 = np.float32
    gt = np.concatenate([Gt[k] for k in range(Nb)], axis=1).astype(f32)   # [128, 1024]
    # block-diagonal W^T so the per-block W-matmul can use the full Mall
    # tile as lhsT (matmul operands must start at base partition 0/32/64)
    wbd = np.zeros((Nb * S, Nb * TbM), np.float64)                         # [128, 1024]
    for k in range(Nb):
        wbd[k * S:(k + 1) * S, k * TbM:(k + 1) * TbM] = Wt[k]
    wbd = wbd.astype(f32)
    mmt = np.concatenate(                                                  # [128, 1024]
        [MM[:, k * TbM:(k + 1) * TbM].T for k in range(Nb)], axis=1
    ).astype(f32)
    phist = PhiStackT.astype(f32)                                          # [16, 128]
    m0r = np.tile(m0.astype(f32)[:, None], (1, BC))                        # [16, 128]
    return gt, wbd, mmt, phist, m0r


# ---------------------------------------------------------------- bass program
def _build_program():
    import concourse.mybir as mybir
    import concourse.tile as tile
    from concourse import bacc

    f32 = mybir.dt.float32
    nc = bacc.Bacc(None, target_bir_lowering=False, debug=False)

    ut_d = nc.dram_tensor("ut", [Nb, TbM, BC], f32, kind="ExternalInput")
    gt_d = nc.dram_tensor("gt", [TbM, Nb * TbM], f32, kind="ExternalInput")
    mmt_d = nc.dram_tensor("mmt", [TbM, Nb * TbM], f32, kind="ExternalInput")
    wt_d = nc.dram_tensor("wt", [Nb * S, Nb * TbM], f32, kind="ExternalInput")
    phist_d = nc.dram_tensor("phist", [S, Nb * S], f32, kind="ExternalInput")
    m0r_d = nc.dram_tensor("m0r", [S, BC], f32, kind="ExternalInput")
    y_d = nc.dram_tensor("y", [BC, TM], f32, kind="ExternalOutput")

    with tile.TileContext(nc) as tc:
        with (
            tc.tile_pool(name="sb", bufs=1) as sb,
            tc.tile_pool(name="ps", bufs=4, space="PSUM") as ps,
        ):
            U = sb.tile([TbM, Nb * BC], f32, tag="U")
            for k in range(Nb):
                nc.sync.dma_start(U[:, k * BC:(k + 1) * BC], ut_d[k])
            Gt = sb.tile([TbM, Nb * TbM], f32, tag="Gt")
            nc.sync.dma_start(Gt[:], gt_d[:])
            Mmt = sb.tile([TbM, Nb * TbM], f32, tag="Mmt")
            nc.sync.dma_start(Mmt[:], mmt_d[:])
            Wt = sb.tile([Nb * S, Nb * TbM], f32, tag="Wt")
            nc.sync.dma_start(Wt[:], wt_d[:])
            Phist = sb.tile([S, Nb * S], f32, tag="Phist")
            nc.sync.dma_start(Phist[:], phist_d[:])
            M0r = sb.tile([S, BC], f32, tag="M0r")
            nc.sync.dma_start(M0r[:], m0r_d[:])

            # block-boundary states: Mall[(k,i), b]
            M_ps = ps.tile([Nb * S, BC], f32, tag="mps")
            nc.tensor.matmul(M_ps[:], Phist[:], M0r[:], start=True, stop=False)
            for k in range(Nb):
                nc.tensor.matmul(
                    M_ps[:],
                    Mmt[:, k * TbM:(k + 1) * TbM],
                    U[:, k * BC:(k + 1) * BC],
                    start=False,
                    stop=(k == Nb - 1),
                )
            Mall = sb.tile([Nb * S, BC], f32, tag="Mall")
            nc.vector.tensor_copy(Mall[:], M_ps[:])

            Ybig = sb.tile([BC, TM], f32, tag="Ybig")
            for k in range(Nb):
                Y_ps = ps.tile([BC, TbM], f32, tag="yps")
                nc.tensor.matmul(
                    Y_ps[:],
                    U[:, k * BC:(k + 1) * BC],
                    Gt[:, k * TbM:(k + 1) * TbM],
                    start=True,
                    stop=False,
                )
                nc.tensor.matmul(
                    Y_ps[:],
                    Mall[:],
                    Wt[:, k * TbM:(k + 1) * TbM],
                    start=False,
                    stop=True,
                )
                nc.vector.tensor_copy(Ybig[:, k * TbM:(k + 1) * TbM], Y_ps[:])

            nc.sync.dma_start(y_d[:], Ybig[:])

    nc.compile()
    return nc


# ---------------------------------------------------------------- entry point
def _make_in_maps(input, F, H, Q, R, initial_state, initial_std_dev):
    gt, wt, mmt, phist, m0r = _device_params(F, H, Q, R, initial_state,
                                             initial_std_dev)
    inp = np.ascontiguousarray(np.asarray(input, np.float32))
    in_maps = []
    for c in range(NCORES):
        chunk = inp[c * BC:(c + 1) * BC].reshape(BC, TM)
        ut = np.stack(
            [np.ascontiguousarray(chunk[:, k * TbM:(k + 1) * TbM].T)
             for k in range(Nb)]
        )
        in_maps.append({"ut": ut, "gt": gt, "mmt": mmt, "wt": wt,
                        "phist": phist, "m0r": m0r})
    return in_maps


def _run(input, F, H, Q, R, initial_state, initial_std_dev, **spmd_kwargs):
    from concourse.bass_utils import run_bass_kernel_spmd

    nc = _build_program()
    in_maps = _make_in_maps(input, F, H, Q, R, initial_state, initial_std_dev)
    res = run_bass_kernel_spmd(nc, in_maps, core_ids=list(range(NCORES)),
                               **spmd_kwargs)
    out = np.concatenate([r["y"] for r in res.results], axis=0)
    return out.reshape(B, T, M).astype(np.float32), res


def kernel(input, F, H, Q, R, initial_state, initial_std_dev):
    out, _ = _run(input, F, H, Q, R, initial_state, initial_std_dev)
    return out


# revision 16
# speedup vs baseline: 1.3497x; 1.3497x over previous
"""Trainium2 Bass kernel for nn_KalmanFilter (B=1024, T=256, S=16, M=4).

Math: the covariance P and Kalman gains K_t never touch the observations,
and P0 = diag(std^2) is identical for every batch element, so K_t (hence
A_t = F(I - K_t H), B_t = F K_t) are shared across the batch.  The filter
collapses to the linear time-varying recurrence

    m_0 = F x0,   m_{t+1} = A_t m_t + B_t u_t,   out_t = H m_t .

Condensing time into Nb=4 blocks of Tb=64 steps (Tb*M = 256):

    out_block_k = U_k^T G_k^T + m_k^T W_k^T        (per-block, batched)
    m_k         = MM_k u_all + Phi(k*Tb, 0) m_0    (block-boundary states)

where U_k is the observation block laid out [Tb*M, Bc].  The small shared
matrices (G_k, W_k, MM, Phi-stack) are data-independent, computed on the
host in float64.  The device does a handful of dense matmuls (float32r,
wide moving operands), data-parallel over the batch on 8 cores (128 batch
rows per core).  Block-state rows are ordered by DESCENDING k so that the
per-chunk m-matmul outputs are contiguous from partition 0 (matmul operands
must start at base partition 0/32/64).
"""

import numpy as np

S, M, T, B = 16, 4, 256, 1024
Tb = 64
Nb = T // Tb          # 4
TbM = Tb * M          # 256
TM = T * M            # 1024
NCORES = 8
BC = B // NCORES      # 128
NC128 = TM // 128     # 8 contraction chunks of 128
# m-matmul lhsT widths per 128-chunk (descending-k row order).  Chunks whose
# block is the last one contribute to no m_k and are dropped; the rest are
# kept at the full Nb*S width so every matmul in the PSUM accumulation group
# covers the same element region (the unused rows are zeros in MM).
MW = [Nb * S if (c // 2) < Nb - 1 else 0 for c in range(NC128)]
MOFF = np.concatenate([[0], np.cumsum(MW)]).tolist()


# ---------------------------------------------------------------- host math
def _gains(F, H, Q, R, x0, std):
    """Shared A_t [T-1,S,S], B_t [T-1,S,M] and m0 [S] in float64."""
    F, H, Q, R = (np.asarray(a, np.float64) for a in (F, H, Q, R))
    x0 = np.asarray(x0, np.float64)
    P = np.diag(np.asarray(std, np.float64) ** 2)
    I = np.eye(S)
    m0 = F @ x0
    P = F @ P @ F.T + Q
    A = np.zeros((T - 1, S, S))
    Bm = np.zeros((T - 1, S, M))
    for s in range(T - 1):
        Sm = H @ P @ H.T + R
        K = P @ H.T @ np.linalg.inv(Sm)
        IKH = I - K @ H
        A[s] = F @ IKH
        Bm[s] = F @ K
        P = IKH @ P @ IKH.T + K @ R @ K.T
        P = F @ P @ F.T + Q
    return A, Bm, m0


def _block_mats(A, Bm, H):
    """G_k^T [Nb,TbM,TbM], W_k^T [Nb,S,TbM], MM [Nb*S,TM], PhiStackT [S,Nb*S].

    MM rows / PhiStackT cols use ascending (k, i) order here; reordering to
    descending-k happens in _device_params.
    """
    H = np.asarray(H, np.float64)
    Gt = np.zeros((Nb, TbM, TbM))
    Wt = np.zeros((Nb, S, TbM))
    MM = np.zeros((Nb * S, TM))
    PhiStackT = np.zeros((S, Nb * S))

    Phi_g = np.eye(S)            # Phi(t0, 0)
    Vg = np.zeros((S, 0))        # [S, t0*M]: Phi(t0, s+1) B_s for s < t0
    for k in range(Nb):
        t0 = k * Tb
        PhiStackT[:, k * S:(k + 1) * S] = Phi_g.T
        MM[k * S:(k + 1) * S, :t0 * M] = Vg
        Phi = np.eye(S)
        V = np.zeros((S, 0))     # within-block propagated B columns
        for dt in range(Tb):
            t = t0 + dt
            Wt[k][:, dt * M:(dt + 1) * M] = (H @ Phi).T
            if dt > 0:
                Gt[k][: dt * M, dt * M:(dt + 1) * M] = (H @ V).T
            if t <= T - 2:
                V = np.concatenate([A[t] @ V, Bm[t]], 1)
                Phi = A[t] @ Phi
        if t0 + Tb <= T - 1:
            Vg = np.concatenate([Phi @ Vg, V], 1)
            Phi_g = Phi @ Phi_g
    return Gt, Wt, MM, PhiStackT


def _device_params(F, H, Q, R, x0, std):
    A, Bm, m0 = _gains(F, H, Q, R, x0, std)
    Gt, Wt, MM, PhiStackT = _block_mats(A, Bm, H)
    f32 = np.float32
    # desc[r] = ascending row index of descending-k row r
    desc = np.concatenate([np.arange(k * S, (k + 1) * S)
                           for k in range(Nb - 1, -1, -1)])

    # G^T halves: gts[c] = Gt[k][h*128:(h+1)*128, :], c = 2k + h  -> [8,128,256]
    gts = np.stack([Gt[c // 2][(c % 2) * 128:(c % 2 + 1) * 128, :]
                    for c in range(2 * Nb)]).astype(f32)
    # block-diagonal W^T with descending-k rows -> [64, 1024]
    wbd = np.zeros((Nb * S, TM))
    for k in range(Nb):
        r0 = (Nb - 1 - k) * S
        wbd[r0:r0 + S, k * TbM:(k + 1) * TbM] = Wt[k]
    wbd = wbd.astype(f32)
    # m-matmul lhsT chunks, concatenated -> [128, sum(MW)]
    MMd = MM[desc]
    mmt = np.concatenate(
        [MMd[:MW[c], c * 128:(c + 1) * 128].T for c in range(NC128) if MW[c]],
        axis=1).astype(f32)
    phist = PhiStackT[:, desc].astype(f32)                 # [16, 64]
    m0r = np.tile(m0.astype(f32)[:, None], (1, BC))        # [16, 128]
    return gts, wbd, mmt, phist, m0r


# ---------------------------------------------------------------- bass program
def _build_program():
    import concourse.mybir as mybir
    import concourse.tile as tile
    from concourse import bacc

    f32 = mybir.dt.float32
    f32r = mybir.dt.float32r
    nc = bacc.Bacc(None, target_bir_lowering=False, debug=False)

    ut_d = nc.dram_tensor("ut", [NC128, 128, BC], f32r, kind="ExternalInput")
    gts_d = nc.dram_tensor("gts", [2 * Nb, 128, TbM], f32r, kind="ExternalInput")
    wbd_d = nc.dram_tensor("wbd", [Nb * S, TM], f32r, kind="ExternalInput")
    mmt_d = nc.dram_tensor("mmt", [128, MOFF[-1]], f32r, kind="ExternalInput")
    phist_d = nc.dram_tensor("phist", [S, Nb * S], f32r, kind="ExternalInput")
    m0r_d = nc.dram_tensor("m0r", [S, BC], f32r, kind="ExternalInput")
    y_d = nc.dram_tensor("y", [BC, TM], f32, kind="ExternalOutput")

    with tile.TileContext(nc) as tc:
        with (
            tc.tile_pool(name="sb", bufs=1) as sb,
            tc.tile_pool(name="ps", bufs=1, space="PSUM") as ps,
        ):
            # ---- loads, spread across the two HWDGE issue engines
            U = sb.tile([128, TM], f32r, tag="U")
            nc.sync.dma_start(U.rearrange("p (c b) -> p c b", c=NC128),
                              ut_d.rearrange("c p b -> p c b"))
            Mmt = sb.tile([128, MOFF[-1]], f32r, tag="Mmt")
            nc.sync.dma_start(Mmt[:], mmt_d[:])
            Phist = sb.tile([S, Nb * S], f32r, tag="Phist")
            nc.sync.dma_start(Phist[:], phist_d[:])
            M0r = sb.tile([S, BC], f32r, tag="M0r")
            nc.sync.dma_start(M0r[:], m0r_d[:])
            Gts = sb.tile([128, 2 * Nb * TbM], f32r, tag="Gts")
            nc.scalar.dma_start(Gts.rearrange("p (c f) -> p c f", c=2 * Nb),
                                gts_d.rearrange("c p f -> p c f"))
            Wbd = sb.tile([Nb * S, TM], f32r, tag="Wbd")
            nc.scalar.dma_start(Wbd[:], wbd_d[:])

            # ---- block-boundary states Mall[(k,i) desc, b]
            M_ps = ps.tile([Nb * S, BC], f32, tag="mps")
            nc.tensor.matmul(M_ps[:], Phist[:],
                             M0r[:], start=True, stop=False)
            live = [c for c in range(NC128) if MW[c]]
            for j, c in enumerate(live):
                nc.tensor.matmul(
                    M_ps[:],
                    Mmt[:, MOFF[c]:MOFF[c] + MW[c]],
                    U[:, c * BC:(c + 1) * BC],
                    start=False,
                    stop=(j == len(live) - 1),
                )
            Mall = sb.tile([Nb * S, BC], f32r, tag="Mall")
            nc.vector.tensor_copy(Mall[:], M_ps[:])

            # ---- per-block Y_k = U_k^T G_k^T + Mall^T Wbd_k, then store
            for k in range(Nb):
                Y_ps = ps.tile([BC, TbM], f32, tag=f"yps{k}")
                for h in range(2):      # contraction half of the G part
                    c = 2 * k + h
                    nc.tensor.matmul(
                        Y_ps[:],
                        U[:, c * BC:(c + 1) * BC],
                        Gts[:, c * TbM:(c + 1) * TbM],
                        start=(h == 0),
                        stop=False,
                    )
                nc.tensor.matmul(
                    Y_ps[:],
                    Mall[:],
                    Wbd[:, k * TbM:(k + 1) * TbM],
                    start=False,
                    stop=True,
                )
                Y_sb = sb.tile([BC, TbM], f32, tag=f"ysb{k}")
                nc.vector.tensor_copy(Y_sb[:], Y_ps[:])
                nc.sync.dma_start(y_d[:, k * TbM:(k + 1) * TbM], Y_sb[:])

    nc.compile()
    return nc


# ---------------------------------------------------------------- entry point
def _make_in_maps(input, F, H, Q, R, initial_state, initial_std_dev):
    gts, wbd, mmt, phist, m0r = _device_params(F, H, Q, R, initial_state,
                                               initial_std_dev)
    inp = np.ascontiguousarray(np.asarray(input, np.float32))
    in_maps = []
    for cid in range(NCORES):
        chunk = inp[cid * BC:(cid + 1) * BC].reshape(BC, TM)
        ut = np.stack(
            [np.ascontiguousarray(chunk[:, c * 128:(c + 1) * 128].T)
             for c in range(NC128)]
        )
        in_maps.append({"ut": ut, "gts": gts, "wbd": wbd, "mmt": mmt,
                        "phist": phist, "m0r": m0r})
    return in_maps


def _run(input, F, H, Q, R, initial_state, initial_std_dev, **spmd_kwargs):
    from concourse.bass_utils import run_bass_kernel_spmd

    nc = _build_program()
    in_maps = _make_in_maps(input, F, H, Q, R, initial_state, initial_std_dev)
    res = run_bass_kernel_spmd(nc, in_maps, core_ids=list(range(NCORES)),
                               **spmd_kwargs)
    out = np.concatenate([r["y"] for r in res.results], axis=0)
    return out.reshape(B, T, M).astype(np.float32), res


def kernel(input, F, H, Q, R, initial_state, initial_std_dev):
    out, _ = _run(input, F, H, Q, R, initial_state, initial_std_dev)
    return out
